# revision 7
# baseline (speedup 1.0000x reference)
"""Distributed attention kernel for TRN2 (8 NeuronCores).

Computes: softmax(sqrt(Dqk) * (x@Wq.T) @ (x@Wk.T).T) @ (x@Wv.T)
for x [8192, 1024], Wq/Wk [256, 1024], Wv [256, 1024], out [8192, 256].

Sharding: rows of x across 8 cores (sequence parallel). Weights replicated.
Each core projects its shard, AllGathers K^T (f32) and V (bf16), then runs
flash-style attention over its 1024 Q rows.

Per-core dataflow:
  - x shard + weights DMA'd in natural layout, PE-transposed to x^T / W^T
    (contract dim on partitions), rounded to float32r for 4x matmul rate.
  - q^T [dqk, nsh] local (scaled by sqrt(dqk) at PSUM eviction),
    k^T [dqk, nsh] -> AllGather -> K^T [dqk, N] assembled in SBUF,
    v [nsh, dv] bf16 -> AllGather -> V [N, dv] in SBUF.
  - per 128-row i-tile: scores chunks in PSUM (f32r matmul), chunk row-max
    on DVE, exp on ACT (PSUM -> SBUF bf16, chunk-max bias, row-sums via
    accum_out), deferred max-correction scaling, P^T via DMA xbar transpose,
    PV matmul bf16, normalize by row-sum, DMA out.
"""

import numpy as np

import concourse.bacc as bacc
import concourse.mybir as mybir
import concourse.tile as tile
from concourse.bass_utils import run_bass_kernel_spmd
from concourse.masks import make_identity

F32 = mybir.dt.float32
F32R = mybir.dt.float32r
BF16 = mybir.dt.bfloat16

N_CORES = 8
N, D, DQK, DV = 8192, 1024, 256, 256
P = 128
CHUNK = 1024  # scores chunk width (2 PSUM banks)


def build(n=N, d=D, dqk=DQK, dv=DV, ncores=N_CORES):
    nsh = n // ncores
    IT = nsh // P       # i-tiles per core
    KT = d // P         # contract tiles for projections
    CT = dqk // P       # dqk tiles
    JT = n // P         # j tiles for PV
    NCH = n // CHUNK    # score chunks per row
    NN = CHUNK // 512   # 512-wide matmuls per chunk
    scale = float(np.sqrt(dqk))

    nc = bacc.Bacc(None, target_bir_lowering=False, num_devices=ncores)

    x_ext = nc.declare_dram_parameter("x", [nsh, d], F32, isOutput=False)
    wq_ext = nc.declare_dram_parameter("Wq", [dqk, d], F32, isOutput=False)
    wk_ext = nc.declare_dram_parameter("Wk", [dqk, d], F32, isOutput=False)
    wv_ext = nc.declare_dram_parameter("Wv", [dv, d], F32, isOutput=False)
    out_ext = nc.declare_dram_parameter("out", [nsh, dv], F32, isOutput=True)

    # DRAM bounce buffers for collectives
    kt_bounce = nc.dram_tensor("kt_bounce", [dqk, nsh], F32R)
    kt_ag = nc.dram_tensor("kt_ag", [ncores * dqk, nsh], F32R, addr_space="Shared")
    v_bounce = nc.dram_tensor("v_bounce", [nsh, dv], BF16)
    v_ag = nc.dram_tensor("v_ag", [n, dv], BF16, addr_space="Shared")

    groups = [list(range(ncores))]

    with tile.TileContext(nc) as tc:
        # ---- long-lived tensors ----
        with tc.tile_pool(name="persist", bufs=1) as pp:
            qt_s = pp.tile([P, CT, nsh], F32R, tag="qt")      # q^T, scaled
            ident = pp.tile([P, P], F32, tag="ident")
            make_identity(nc, ident[:])

            # ================= Phase A: projections =================
            with (
                tc.tile_pool(name="phA", bufs=1) as pa,
                tc.tile_pool(name="phA_psum", bufs=1, space="PSUM") as paps,
            ):
                x_nat = pa.tile([P, IT, d], F32, tag="xnat")
                nc.sync.dma_start(
                    x_nat[:], x_ext.ap().rearrange("(it p) d -> p it d", p=P)
                )
                w_nat = pa.tile([P, 3 * CT, d], F32, tag="wnat")
                for wi, w_ext in enumerate((wq_ext, wk_ext, wv_ext)):
                    nc.sync.dma_start(
                        w_nat[:, wi * CT:(wi + 1) * CT, :],
                        w_ext.ap().rearrange("(ct p) d -> p ct d", p=P),
                    )

                xt_s = pa.tile([P, KT, nsh], F32R, tag="xt")
                wt_s = pa.tile([P, 3 * KT, dqk], F32R, tag="wt")

                # PE-transpose x: block (kt, it): x[i, kt*P+p] -> xT[p, kt, i]
                for kt in range(KT):
                    for it in range(IT):
                        tp = paps.tile([P, P], F32, tag="tp", bufs=2)
                        nc.tensor.transpose(
                            tp[:], x_nat[:, it, kt * P:(kt + 1) * P], ident[:]
                        )
                        nc.vector.tensor_copy(
                            xt_s[:, kt, it * P:(it + 1) * P], tp[:]
                        )
                # PE-transpose weights: W[c, kt*P+p] -> WT[p, wi*KT+kt, c]
                for wi in range(3):
                    for kt in range(KT):
                        for ct in range(CT):
                            tp = paps.tile([P, P], F32, tag="tp", bufs=2)
                            nc.tensor.transpose(
                                tp[:],
                                w_nat[:, wi * CT + ct, kt * P:(kt + 1) * P],
                                ident[:],
                            )
                            nc.vector.tensor_copy(
                                wt_s[:, wi * KT + kt, ct * P:(ct + 1) * P], tp[:]
                            )

                # q^T, k^T: out[c, i] — lhsT = WT block [d_p, c], rhs = xT [d_p, i]
                kt_loc = pa.tile([P, CT, nsh], F32R, tag="ktloc")
                ich_w = min(512, nsh)
                n_ich = nsh // ich_w
                for ct in range(CT):
                    for ich in range(n_ich):
                        psq = paps.tile([P, ich_w], F32, tag="psq", bufs=2)
                        psk = paps.tile([P, ich_w], F32, tag="psk", bufs=2)
                        for kt in range(KT):
                            nc.tensor.matmul(
                                psq[:],
                                wt_s[:, 0 * KT + kt, ct * P:(ct + 1) * P],
                                xt_s[:, kt, ich * ich_w:(ich + 1) * ich_w],
                                start=(kt == 0),
                                stop=(kt == KT - 1),
                            )
                        for kt in range(KT):
                            nc.tensor.matmul(
                                psk[:],
                                wt_s[:, 1 * KT + kt, ct * P:(ct + 1) * P],
                                xt_s[:, kt, ich * ich_w:(ich + 1) * ich_w],
                                start=(kt == 0),
                                stop=(kt == KT - 1),
                            )
                        nc.vector.tensor_scalar_mul(
                            qt_s[:, ct, ich * ich_w:(ich + 1) * ich_w], psq[:], scale
                        )
                        nc.vector.tensor_copy(
                            kt_loc[:, ct, ich * ich_w:(ich + 1) * ich_w], psk[:]
                        )
                # v natural: out[i, cv] — lhsT = xT block [d_p, i], rhs = WvT [d_p, cv]
                v_loc = pa.tile([P, IT, dv], BF16, tag="vloc")
                for it in range(IT):
                    psv = paps.tile([P, dv], F32, tag="psv", bufs=2)
                    for kt in range(KT):
                        nc.tensor.matmul(
                            psv[:],
                            xt_s[:, kt, it * P:(it + 1) * P],
                            wt_s[:, 2 * KT + kt, :dqk],
                            start=(kt == 0),
                            stop=(kt == KT - 1),
                        )
                    nc.vector.tensor_copy(v_loc[:, it, :], psv[:])

                # stage shards to DRAM for the collectives
                nc.sync.dma_start(
                    kt_bounce.ap().rearrange("(ct p) i -> p ct i", p=P), kt_loc[:]
                )
                nc.sync.dma_start(
                    v_bounce.ap().rearrange("(it p) c -> p it c", p=P), v_loc[:]
                )

            # ================= AllGather K^T and V =================
            phb_cm = tc.tile_pool(name="phB", bufs=1)
            phb = phb_cm.__enter__()
            kt_full = phb.tile([P, CT, n], F32R, tag="ktf", name="kt_full")
            v_s = phb.tile([P, JT, dv], BF16, tag="vs", name="v_s")

            nc.gpsimd.collective_compute(
                "AllGather",
                mybir.AluOpType.bypass,
                replica_groups=groups,
                ins=[kt_bounce.ap().bitcast(F32).opt()],
                outs=[kt_ag.ap().bitcast(F32).opt()],
            )
            nc.gpsimd.collective_compute(
                "AllGather",
                mybir.AluOpType.bypass,
                replica_groups=groups,
                ins=[v_bounce.ap().opt()],
                outs=[v_ag.ap().opt()],
            )

            # assemble K^T [dqk, n]: kt_ag[(r ct p), i] -> kt_full[p, ct, r*nsh+i]
            for ct in range(CT):
                nc.sync.dma_start(
                    kt_full[:, ct, :].rearrange("p (r i) -> p r i", r=ncores),
                    kt_ag.ap().rearrange("(r c) i -> c r i", c=dqk)[
                        ct * P:(ct + 1) * P
                    ],
                )
            # assemble V [n, dv]: v_ag[(jt p), c] -> v_s[p, jt, c]
            nc.sync.dma_start(
                v_s[:], v_ag.ap().rearrange("(jt p) c -> p jt c", p=P)
            )

            # ================= Main attention loop =================
            with (
                tc.tile_pool(name="mainA", bufs=2) as ma,
                tc.tile_pool(name="scores_psum", bufs=3, space="PSUM") as sps,
                tc.tile_pool(name="out_psum", bufs=2, space="PSUM") as ops,
            ):
                for it in range(IT):
                    p_t = ma.tile([P, n], BF16, tag="p")
                    pt_t = ma.tile([P, JT, P], BF16, tag="pt")
                    mneg = ma.tile([P, NCH], F32, tag="mneg")
                    sig = ma.tile([P, NCH], F32, tag="sig")
                    alpha = ma.tile([P, NCH], F32, tag="alpha")
                    mst = ma.tile([P, 1], F32, tag="mst")
                    asig = ma.tile([P, NCH], F32, tag="asig")
                    rs = ma.tile([P, 1], F32, tag="rs")
                    rinv = ma.tile([P, 1], F32, tag="rinv")
                    o_t = ma.tile([P, dv], F32, tag="o")

                    for ch in range(NCH):
                        ps = sps.tile([P, CHUNK], F32, tag="s")
                        for nn in range(NN):
                            for ct in range(CT):
                                nc.tensor.matmul(
                                    ps[:, nn * 512:(nn + 1) * 512],
                                    qt_s[:, ct, it * P:(it + 1) * P],
                                    kt_full[
                                        :, ct,
                                        ch * CHUNK + nn * 512:
                                        ch * CHUNK + (nn + 1) * 512,
                                    ],
                                    start=(ct == 0),
                                    stop=(ct == CT - 1),
                                )
                        # -max of chunk (negated for exp bias)
                        nc.vector.reduce_max(
                            mneg[:, ch:ch + 1], ps[:],
                            axis=mybir.AxisListType.X, negate=True,
                        )
                        # P_ch = exp(s - m_ch), row sums via accumulator
                        nc.scalar.activation(
                            p_t[:, ch * CHUNK:(ch + 1) * CHUNK],
                            ps[:],
                            mybir.ActivationFunctionType.Exp,
                            bias=mneg[:, ch:ch + 1],
                            scale=1.0,
                            accum_out=sig[:, ch:ch + 1],
                        )

                    # alpha_ch = exp(m_ch - m*) = exp(mneg* - mneg_ch)
                    nc.vector.tensor_reduce(
                        mst[:], mneg[:], axis=mybir.AxisListType.X,
                        op=mybir.AluOpType.min,
                    )
                    nc.scalar.activation(
                        alpha[:], mneg[:],
                        mybir.ActivationFunctionType.Exp,
                        bias=mst[:], scale=-1.0,
                    )
                    nc.vector.tensor_mul(asig[:], alpha[:], sig[:])
                    nc.vector.reduce_sum(rs[:], asig[:], axis=mybir.AxisListType.X)
                    nc.vector.reciprocal(rinv[:], rs[:])

                    for ch in range(NCH):
                        nc.vector.tensor_scalar_mul(
                            p_t[:, ch * CHUNK:(ch + 1) * CHUNK],
                            p_t[:, ch * CHUNK:(ch + 1) * CHUNK],
                            alpha[:, ch:ch + 1],
                        )
                        # P^T chunk via DMA xbar transpose:
                        # p[i, ch*CHUNK + jt*P + p'] -> pt[p', ch*(CHUNK/P)+jt, i]
                        nc.sync.dma_start_transpose(
                            pt_t[:, ch * (CHUNK // P):(ch + 1) * (CHUNK // P), :],
                            p_t[:, ch * CHUNK:(ch + 1) * CHUNK],
                        )

                    po = ops.tile([P, dv], F32, tag="po")
                    for jt in range(JT):
                        nc.tensor.matmul(
                            po[:], pt_t[:, jt, :], v_s[:, jt, :],
                            start=(jt == 0), stop=(jt == JT - 1),
                        )
                    nc.vector.tensor_scalar_mul(o_t[:], po[:], rinv[:])
                    nc.sync.dma_start(
                        out_ext.ap().rearrange("(it p) c -> p it c", p=P)[:, it, :],
                        o_t[:],
                    )

            phb_cm.__exit__(None, None, None)

    nc.finalize()
    return nc


_NC_CACHE = {}


def _get_nc(key):
    if key not in _NC_CACHE:
        n, d, dqk, dv, ncores = key
        _NC_CACHE[key] = build(n=n, d=d, dqk=dqk, dv=dv, ncores=ncores)
    return _NC_CACHE[key]


def run(x, Wq, Wk, Wv, trace=False):
    n, d = x.shape
    dqk = Wq.shape[0]
    dv = Wv.shape[0]
    ncores = N_CORES
    nsh = n // ncores
    nc = _get_nc((n, d, dqk, dv, ncores))

    x = np.ascontiguousarray(x, dtype=np.float32)
    Wq = np.ascontiguousarray(Wq, dtype=np.float32)
    Wk = np.ascontiguousarray(Wk, dtype=np.float32)
    Wv = np.ascontiguousarray(Wv, dtype=np.float32)

    in_maps = [
        {"x": x[r * nsh:(r + 1) * nsh], "Wq": Wq, "Wk": Wk, "Wv": Wv}
        for r in range(ncores)
    ]
    res = run_bass_kernel_spmd(
        nc, in_maps, core_ids=list(range(ncores)), trace=trace
    )
    out = np.concatenate([res.results[r]["out"] for r in range(ncores)], axis=0)
    return out, res


def kernel(x, Wq, Wk, Wv):
    out, _ = run(x, Wq, Wk, Wv)
    return out


if __name__ == "__main__":
    rng = np.random.default_rng(0)
    n = 1024
    x = rng.standard_normal((n, D), dtype=np.float32)
    s = 1.0 / np.sqrt(D)
    Wq = rng.uniform(-s, s, (DQK, D)).astype(np.float32)
    Wk = rng.uniform(-s, s, (DQK, D)).astype(np.float32)
    Wv = rng.uniform(-s, s, (DV, D)).astype(np.float32)
    out, _ = run(x, Wq, Wk, Wv)
    q = x @ Wq.T
    k = x @ Wk.T
    v = x @ Wv.T
    sc = np.sqrt(DQK) * (q @ k.T)
    a = np.exp(sc - sc.max(-1, keepdims=True))
    a /= a.sum(-1, keepdims=True)
    ref = a @ v
    err = np.linalg.norm(out - ref) / np.linalg.norm(ref)
    print(f"small-n HW check rel err: {err:.3e}")
